# revision 9
# baseline (speedup 1.0000x reference)
"""Trainium2 Bass kernel for ComplexAttention.

Math (per (b,t) pair):
    cur2 = [cur_r, cur_i]                       # [2D]
    Q    = cur2 @ qW + qb                       # [D]
    K_s  = H_s @ kW + kb ; V_s = H_s @ vW + vb  # H = [hist_r, hist_i]  [S, 2D]
    sc_s = (Q . K_s) * scale * conf
    w    = softmax(sc) ; ctx = sum_s w_s V_s
    out  = cur + 0.1 * ctx (complex)

Rewrites (exact):
    Q . K_s = (cur2 @ (qW kW^T) + qb kW^T) . H_s  + (Q . kb)  [const in s, dropped]
    ctx = (sum_s w_s H_s) @ vW + vb              (sum_s w_s = 1)
so only two small contractions touch H; the projections collapse to
    Qk  = conf*scale*(cur2 @ Wqk + bqk)   (Wqk = qW kW^T host-folded, conf in
                                           the PSUM->SBUF copy scale)
    out = (hbarT^T @ vW)*0.1 + (cur + 0.1*vb)    (vb/0.1 host-folded into cur)

Per 128-pair batch, pairs are processed in groups of 4 stacked (j,s) on the
128 partitions.  Per group:
    qkr    = replicate Qk rows 32x (PE, r32 one-hot matmul)
    scores = sum_e H*qkr  (one fused DVE scalar_tensor_tensor with accum_out)
    wd     = m4*exp*inv   (softmax weights, normalization folded in)
    hbT    = H^T @ wd     (PE, 8 e-chunks -> [128, 4] each: hbar TRANSPOSED,
                           so no separate transpose stage is needed)
hbT accumulates per sub-batch in PSUM, one ACT copy -> fp16 SBUF, and the
final ctx matmul consumes it directly as lhsT.

Sharding: data-parallel over the 4096 (b,t) pairs, 512 per core.
"""

import os
import sys

import numpy as np

os.environ.setdefault("MYCRO_LOCAL_CACHE", "1")

try:
    import concourse.bass as bass
except ImportError:  # pragma: no cover
    sys.path.insert(0, "/opt/trn_rl_repo")
    import concourse.bass as bass

import concourse.mybir as mybir
import concourse.tile as tile
from concourse import bacc
from concourse.bass_utils import run_bass_kernel_spmd

F32 = mybir.dt.float32
F32R = mybir.dt.float32r
F16 = mybir.dt.float16
AX = mybir.AluOpType
AF = mybir.ActivationFunctionType

B, T, S, D = 4, 1024, 32, 512
D2 = 2 * D  # 1024, concat(real, imag) feature dim
E = 2 * D   # 1024, history feature dim
N_CORES = 8
PAIRS = B * T
SCALE = float(D) ** -0.5


def build(ppc: int, bench_loop: int = 0) -> bass.Bass:
    """Build the per-core SPMD program for `ppc` pairs per core.

    bench_loop > 0 wraps the body in a hardware repeat loop for
    loop-differencing timing (NTFF profiling is unavailable here).
    """
    assert ppc % 128 == 0
    nb = ppc // 128      # batches of 128 pairs

    nc = bacc.Bacc("TRN2", target_bir_lowering=False)

    hist_r = nc.declare_dram_parameter("hist_real", [ppc, S, D], F32, isOutput=False)
    hist_i = nc.declare_dram_parameter("hist_imag", [ppc, S, D], F32, isOutput=False)
    cur_r = nc.declare_dram_parameter("cur_r", [ppc, D], F32, isOutput=False)
    cur_i = nc.declare_dram_parameter("cur_i", [ppc, D], F32, isOutput=False)
    cur2t = nc.declare_dram_parameter("cur2t", [D2, ppc], F16, isOutput=False)
    wqk = nc.declare_dram_parameter("wqk", [D2, E], F16, isOutput=False)
    bqk = nc.declare_dram_parameter("bqk", [1, E], F16, isOutput=False)
    vw = nc.declare_dram_parameter("vw", [E, E], F16, isOutput=False)
    confb = nc.declare_dram_parameter("confb", [128, nb], F32, isOutput=False)
    r32 = nc.declare_dram_parameter("r32", [128, 8, 128], F16, isOutput=False)
    m4 = nc.declare_dram_parameter("m4", [128, 4], F32, isOutput=False)
    rep4 = nc.declare_dram_parameter("rep4", [4, 128], F32, isOutput=False)
    ones1 = nc.declare_dram_parameter("ones1", [1, 128], F16, isOutput=False)
    out = nc.declare_dram_parameter("out", [ppc, D, 2], F32, isOutput=True)

    from contextlib import ExitStack

    with tile.TileContext(nc) as tc, ExitStack() as es:
        ec = es.enter_context
        cpool = ec(tc.tile_pool(name="const", bufs=1))
        wqpool = ec(tc.tile_pool(name="wq", bufs=1))
        vwpool = ec(tc.tile_pool(name="vw", bufs=1))
        hpool = ec(tc.tile_pool(name="h", bufs=4))
        qkpool = ec(tc.tile_pool(name="qk", bufs=2))
        sinkpool = ec(tc.tile_pool(name="sink", bufs=2))
        smpool = ec(tc.tile_pool(name="sm", bufs=2))
        wdpool = ec(tc.tile_pool(name="wd", bufs=4))
        invpool = ec(tc.tile_pool(name="inv", bufs=2))
        hbtpool = ec(tc.tile_pool(name="hbt", bufs=2))
        curpool = ec(tc.tile_pool(name="cur", bufs=2))
        outpool = ec(tc.tile_pool(name="outp", bufs=2))
        ps_sh = ec(tc.tile_pool(name="ps_sh", bufs=2, space="PSUM"))
        ps_rep = ec(tc.tile_pool(name="ps_rep", bufs=2, space="PSUM"))
        ps_hbt = ec(tc.tile_pool(name="ps_hbt", bufs=2, space="PSUM"))
        del es
        from contextlib import nullcontext

        loop_cm = tc.For_i(0, bench_loop, 1) if bench_loop > 0 else nullcontext()
        with loop_cm:
            body(nc, tc, ppc, nb, locals())

    nc.compile()
    return nc


def body(nc, tc, ppc, nb, pools):
    (cpool, wqpool, vwpool, hpool, qkpool, sinkpool, smpool, wdpool, invpool,
     hbtpool, curpool, outpool, ps_sh, ps_rep, ps_hbt) = (
        pools["cpool"], pools["wqpool"], pools["vwpool"], pools["hpool"],
        pools["qkpool"], pools["sinkpool"], pools["smpool"], pools["wdpool"],
        pools["invpool"], pools["hbtpool"], pools["curpool"], pools["outpool"],
        pools["ps_sh"], pools["ps_rep"], pools["ps_hbt"])
    hist_r, hist_i = pools["hist_r"], pools["hist_i"]
    cur_r, cur_i, cur2t = pools["cur_r"], pools["cur_i"], pools["cur2t"]
    wqk, bqk, vw, confb = pools["wqk"], pools["bqk"], pools["vw"], pools["confb"]
    r32, m4, rep4, ones1, out = (pools["r32"], pools["m4"], pools["rep4"],
                                 pools["ones1"], pools["out"])
    if True:
        # ---- constants / weights resident in SBUF ----
        m4_t = cpool.tile([128, 4], F32)
        nc.sync.dma_start(out=m4_t[:], in_=m4[:])
        rep4_t = cpool.tile([4, 128], F32)
        nc.sync.dma_start(out=rep4_t[:], in_=rep4[:])
        ones_t = cpool.tile([1, 128], F16)
        nc.sync.dma_start(out=ones_t[:], in_=ones1[:])
        bqk_t = cpool.tile([1, E], F16)
        nc.sync.dma_start(out=bqk_t[:], in_=bqk[:])
        confb_t = cpool.tile([128, nb], F32)
        nc.sync.dma_start(out=confb_t[:], in_=confb[:])
        r32_t = cpool.tile([128, 8, 128], F16)
        nc.sync.dma_start(out=r32_t[:], in_=r32[:])
        c2t_t = cpool.tile([128, 8, ppc], F16)
        nc.sync.dma_start(
            out=c2t_t[:], in_=cur2t[:].rearrange("(k p) n -> p k n", p=128)
        )
        wqk_t = wqpool.tile([128, 8, E], F16)
        nc.sync.dma_start(
            out=wqk_t[:], in_=wqk[:].rearrange("(k p) e -> p k e", p=128)
        )
        vw_t = vwpool.tile([128, 8, E], F16)

        for b in range(nb):
            cur_t = curpool.tile([128, 2, D], F32)
            nc.sync.dma_start(out=cur_t[:, 0, :], in_=cur_r[128 * b : 128 * (b + 1), :])
            nc.sync.dma_start(out=cur_t[:, 1, :], in_=cur_i[128 * b : 128 * (b + 1), :])

            # phase A: Qk = conf*scale*(cur2 @ Wqk + bqk) -> fp16 [128, E]
            qk_t = qkpool.tile([128, E], F16, tag="qk")
            for h in range(2):
                ps = ps_sh.tile([128, 512], F32, tag="sh")
                for k in range(8):
                    nc.tensor.matmul(
                        ps[:],
                        lhsT=c2t_t[:, k, 128 * b : 128 * (b + 1)],
                        rhs=wqk_t[:, k, 512 * h : 512 * (h + 1)],
                        start=(k == 0),
                        stop=False,
                    )
                nc.tensor.matmul(
                    ps[:], lhsT=ones_t[:], rhs=bqk_t[:, 512 * h : 512 * (h + 1)],
                    start=False, stop=True,
                )
                nc.scalar.activation(
                    qk_t[:, 512 * h : 512 * (h + 1)], ps[:], AF.Copy,
                    scale=confb_t[:, b : b + 1],
                )
            # relayout to [32, 4, E] so the replication matmul reads base 0
            qk4 = qkpool.tile([32, 4, E], F16, tag="qk4")
            for blk in range(4):
                nc.gpsimd.dma_start(
                    out=qk4[:, blk, :], in_=qk_t[32 * blk : 32 * (blk + 1), :]
                )

            hbt_b = hbtpool.tile([128, 8, 128], F16)  # hbarT, e-chunk major

            h_tiles, sc, ex, iv = {}, {}, {}, {}

            def stage1(sb, b=b, qk4=qk4, h_tiles=h_tiles, sc=sc):
                """H DMA + Qk replication + fused scores."""
                hts = []
                for half in range(2):
                    h_t = hpool.tile([128, 4, E], F32R, tag="h")
                    p0 = 4 * (32 * b + 8 * sb + 4 * half)
                    nc.sync.dma_start(
                        out=h_t[:, :, 0:D],
                        in_=hist_r[p0 : p0 + 16].bitcast(F32R).rearrange(
                            "(gl j) s d -> (j s) gl d", j=4
                        ),
                    )
                    nc.sync.dma_start(
                        out=h_t[:, :, D:E],
                        in_=hist_i[p0 : p0 + 16].bitcast(F32R).rearrange(
                            "(gl j) s d -> (j s) gl d", j=4
                        ),
                    )
                    hts.append(h_t)
                h_tiles[sb] = hts
                scores8 = smpool.tile([128, 8], F32, tag="scores")
                for gl in range(8):
                    qkr = ps_rep.tile([128, E], F32, tag="rep")
                    for h in range(2):
                        nc.tensor.matmul(
                            qkr[:, 512 * h : 512 * (h + 1)],
                            lhsT=r32_t[0:32, gl, :],
                            rhs=qk4[:, sb, 512 * h : 512 * (h + 1)],
                            start=True, stop=True,
                        )
                    sink = sinkpool.tile([128, E], F16, tag="sink")
                    nc.vector.scalar_tensor_tensor(
                        out=sink[:],
                        in0=hts[gl // 4][:, gl % 4, :].bitcast(F32),
                        scalar=1.0,
                        in1=qkr[:],
                        op0=AX.mult,
                        op1=AX.mult,
                        accum_out=scores8[:, gl : gl + 1],
                    )
                sc[sb] = scores8

            def stage2(sb, sc=sc, ex=ex, iv=iv):
                """softmax aux: exp, denominators, replicated reciprocals."""
                exp8 = smpool.tile([128, 8], F32, tag="exp")
                nc.scalar.activation(exp8[:], sc[sb][:], AF.Exp)
                dn4 = ps_sh.tile([4, 8], F32, tag="sh")
                nc.tensor.matmul(dn4[:], lhsT=m4_t[:], rhs=exp8[:], start=True, stop=True)
                inv4 = invpool.tile([4, 8], F32, tag="inv4")
                nc.vector.reciprocal(inv4[:], dn4[:])
                ivp = ps_sh.tile([128, 8], F32, tag="sh")
                nc.tensor.matmul(ivp[:], lhsT=rep4_t[:], rhs=inv4[:], start=True, stop=True)
                invs = invpool.tile([128, 8], F32, tag="invs")
                nc.scalar.activation(invs[:], ivp[:], AF.Copy)
                ex[sb], iv[sb] = exp8, invs

            def stage3(sb, h_tiles=h_tiles, ex=ex, iv=iv, hbt_b=hbt_b):
                """normalized softmax weights + transposed weighted H sum."""
                hbt_ps = ps_hbt.tile([128, 8, 32], F32, tag="hbt")
                for gl in range(8):
                    wd = wdpool.tile([128, 4], F32R, tag="wd")
                    nc.vector.tensor_scalar(
                        wd[:], m4_t[:], ex[sb][:, gl : gl + 1],
                        iv[sb][:, gl : gl + 1], AX.mult, op1=AX.mult,
                    )
                    for c in range(8):
                        nc.tensor.matmul(
                            hbt_ps[:, c, 4 * gl : 4 * (gl + 1)],
                            lhsT=h_tiles[sb][gl // 4][:, gl % 4, 128 * c : 128 * (c + 1)],
                            rhs=wd[:],
                            start=True, stop=True,
                        )
                # 0.1 applied here; fp16 convert; e-major layout for ctx lhsT
                nc.scalar.activation(
                    hbt_b[:, :, 32 * sb : 32 * (sb + 1)], hbt_ps[:], AF.Copy,
                    scale=0.1,
                )

            # software pipelining: scores of sb+1 overlap the weighted-sum of sb
            stage1(0)
            stage2(0)
            stage1(1)
            if b == 0:
                nc.sync.dma_start(
                    out=vw_t[:], in_=vw[:].rearrange("(k p) e -> p k e", p=128)
                )
            stage3(0)
            stage2(1)
            stage1(2)
            stage3(1)
            stage2(2)
            stage1(3)
            stage3(2)
            stage2(3)
            stage3(3)

            # ctx = hbarT^T @ vW (0.1/norm already folded) + cur'
            out_t = outpool.tile([128, D, 2], F32)
            for h2 in range(2):
                cps = ps_sh.tile([128, 512], F32, tag="sh")
                for c in range(8):
                    nc.tensor.matmul(
                        cps[:],
                        lhsT=hbt_b[:, c, :],
                        rhs=vw_t[:, c, 512 * h2 : 512 * (h2 + 1)],
                        start=(c == 0),
                        stop=(c == 7),
                    )
                nc.vector.tensor_tensor(
                    out=out_t[:, :, h2], in0=cps[:], in1=cur_t[:, h2, :], op=AX.add
                )
            nc.sync.dma_start(
                out=out[:].rearrange("(bb p) d two -> p bb d two", p=128)[:, b],
                in_=out_t[:],
            )


_CACHE: dict[int, bass.Bass] = {}


def get_nc(ppc: int) -> bass.Bass:
    if ppc not in _CACHE:
        _CACHE[ppc] = build(ppc)
    return _CACHE[ppc]


def make_const_inputs():
    r32_v = np.zeros((8, 32, 128), np.float16)
    for v in range(8):
        for j in range(4):
            r32_v[v, 4 * v + j, 32 * j : 32 * (j + 1)] = 1.0
    # tiled into each 32-partition block so lhsT base matches the rhs base
    r32_h = np.ascontiguousarray(np.tile(r32_v.transpose(1, 0, 2), (4, 1, 1)))
    m4_h = np.zeros((128, 4), np.float32)
    rep4_h = np.zeros((4, 128), np.float32)
    for j in range(4):
        m4_h[32 * j : 32 * (j + 1), j] = 1.0
        rep4_h[j, 32 * j : 32 * (j + 1)] = 1.0
    ones_h = np.ones((1, 128), np.float16)
    return r32_h, m4_h, rep4_h, ones_h


def host_prep(hist_real, hist_imag, current_real, current_imag, confidence,
              qW, qb, kW, kb, vW, vb, ppc):
    """Shared host-side folding + per-core input maps."""
    f = lambda x: np.ascontiguousarray(np.asarray(x, dtype=np.float32))
    hist_real, hist_imag = f(hist_real), f(hist_imag)
    current_real, current_imag = f(current_real), f(current_imag)
    confidence = f(confidence)
    qW, qb, kW, kb, vW, vb = f(qW), f(qb), f(kW), f(kb), f(vW), f(vb)

    n_cores = (B * T) // ppc
    nb = ppc // 128
    wqk_h = np.ascontiguousarray(qW @ kW.T).astype(np.float16)   # [D2, E]
    bqk_h = (qb @ kW.T).reshape(1, E).astype(np.float16)
    vw_h = vW.astype(np.float16)
    r32_h, m4_h, rep4_h, ones_h = make_const_inputs()

    hr = hist_real.reshape(B * T, S, D)
    hi = hist_imag.reshape(B * T, S, D)
    cr = current_real.reshape(B * T, D)
    ci = current_imag.reshape(B * T, D)
    cf = confidence.reshape(B * T)
    # vb and the 0.1 ctx scale folded into the residual input
    crv = cr + 0.1 * vb[:D]
    civ = ci + 0.1 * vb[D:]

    in_maps = []
    for c in range(n_cores):
        sl = slice(c * ppc, (c + 1) * ppc)
        cur2t_h = np.ascontiguousarray(
            np.concatenate([cr[sl], ci[sl]], axis=1).T
        ).astype(np.float16)                              # [D2, ppc]
        confb_h = np.ascontiguousarray(
            cf[sl].reshape(nb, 128).T * SCALE
        )                                                 # [128, nb]
        in_maps.append({
            "hist_real": hr[sl],
            "hist_imag": hi[sl],
            "cur_r": np.ascontiguousarray(crv[sl]),
            "cur_i": np.ascontiguousarray(civ[sl]),
            "cur2t": cur2t_h,
            "wqk": wqk_h,
            "bqk": bqk_h,
            "vw": vw_h,
            "confb": confb_h,
            "r32": r32_h,
            "m4": m4_h,
            "rep4": rep4_h,
            "ones1": ones_h,
        })
    return in_maps


def kernel(hist_real, hist_imag, current_real, current_imag, confidence,
           qW, qb, kW, kb, vW, vb):
    ppc = PAIRS // N_CORES
    nc = get_nc(ppc)
    in_maps = host_prep(hist_real, hist_imag, current_real, current_imag,
                        confidence, qW, qb, kW, kb, vW, vb, ppc)
    res = run_bass_kernel_spmd(nc, in_maps, list(range(N_CORES))).results
    out = np.concatenate([res[c]["out"] for c in range(N_CORES)], axis=0)
    return out.view(np.complex64)[..., 0].reshape(B, T, D)
